# revision 4
# baseline (speedup 1.0000x reference)
"""GPT-2 attention block (B=2, S=2048, E=1024, H=16) on 8 TRN2 NeuronCores.

Sharding: 8-way tensor parallel over heads (2 heads/core) for the qkv
projection and attention; AllToAll reshards attention output from
head-sharded to token-sharded so each core computes the c_proj output for
its 512-token chunk with full contraction. Matmuls run in float32r
(full-rate PE, ~1.4e-4 rel err); accumulation is fp32 in PSUM.

Per-core dataflow:
  x [4096,1024] --PE transpose--> hT [1024,4096]
  qT = Wq^T hT + bq   [128,4096]   (ACT evac fused bias)
  kT = Wk^T hT + bk   [128,4096]
  V  = hT^T Wv + bv   [4096, 2, 65] (65th col = ones for softmax row sums)
  per (batch, head, 512-wide q tile):
    S^T tile = K Q^T / 8 ; P^T = exp(S^T) (no max subtraction: |logits/8|<~3)
    O'^T[65,512] = [V|1]^T P^T accumulated over 16 k tiles (row 64 = sums)
    O^T = O'^T[0:64] * broadcast(1/sums)
  AllToAll -> each core holds all 1024 attention channels for its token chunk
  y = O[tok chunk] @ Wp + bp  -> out [512, 1024]
"""

import sys

if "/opt/trn_rl_repo" not in sys.path:
    sys.path.insert(0, "/opt/trn_rl_repo")

import numpy as np

import concourse.bass as bass  # noqa: F401  (engine handles via nc)
import concourse.mybir as mybir
from concourse import bacc, tile
from concourse.bass_utils import run_bass_kernel_spmd
from concourse.masks import make_identity

F32 = mybir.dt.float32
F32R = mybir.dt.float32r
AF = mybir.ActivationFunctionType

B, S, E, H = 2, 2048, 1024, 16
D = E // H            # 64
NC = 8                # cores
HPC = H // NC         # 2 heads per core
FPC = HPC * D         # 128 per-core q/k/v feature count
T = B * S             # 4096 tokens, batch-major
TC = T // NC          # 512 output tokens per core
NTT = T // 128        # 32 token tiles of 128
NST = T // 512        # 8 token supertiles of 512
NEC = E // 128        # 8 contraction chunks
KT_PER_B = S // 128   # 16 k tiles per batch
QT_PER_B = S // 512   # 4 q tiles per batch


def build_nc():
    nc = bacc.Bacc("TRN2", target_bir_lowering=False, debug=False, num_devices=NC)

    x_ext = nc.dram_tensor("x", [T, E], F32, kind="ExternalInput")
    wq_ext = nc.dram_tensor("wq", [E, FPC], F32R, kind="ExternalInput")
    wk_ext = nc.dram_tensor("wk", [E, FPC], F32R, kind="ExternalInput")
    wv_ext = nc.dram_tensor("wv", [E, FPC], F32R, kind="ExternalInput")
    wp_ext = nc.dram_tensor("wp", [E, E], F32R, kind="ExternalInput")
    bq_ext = nc.dram_tensor("bq", [FPC], F32, kind="ExternalInput")
    bk_ext = nc.dram_tensor("bk", [FPC], F32, kind="ExternalInput")
    bv_ext = nc.dram_tensor("bv", [FPC], F32R, kind="ExternalInput")
    bp_ext = nc.dram_tensor("bp", [E], F32R, kind="ExternalInput")
    out_ext = nc.dram_tensor("out", [TC, E], F32, kind="ExternalOutput")

    # AllToAll bounce buffers: chunk/block j is [128 channels, 512 tokens].
    o_loc = nc.dram_tensor("o_loc", [NC, FPC, TC], F32R)
    o_gat = nc.dram_tensor("o_gat", [NC, FPC, TC], F32R)

    with tile.TileContext(nc) as tc:
        with (
            tc.tile_pool(name="const", bufs=1) as cpool,
            tc.tile_pool(name="wqkv", bufs=1) as wpool,
            tc.tile_pool(name="attn_persist", bufs=1) as apool,
        ):
            ident = cpool.tile([128, 128], F32)
            make_identity(nc, ident[:])
            ones_f32 = cpool.tile([128, 128], F32)
            nc.vector.memset(ones_f32[:], 1.0)
            ones_r = cpool.tile([1, 128], F32R)
            nc.vector.tensor_copy(ones_r[:], ones_f32[0:1, :])
            bq_sb = cpool.tile([128, 1], F32)
            bk_sb = cpool.tile([128, 1], F32)
            bv_sb = cpool.tile([1, FPC], F32R)
            bp_sb = cpool.tile([1, E], F32R)
            nc.sync.dma_start(out=bq_sb[:], in_=bq_ext.ap().rearrange("(p a) -> p a", p=FPC))
            nc.sync.dma_start(out=bk_sb[:], in_=bk_ext.ap().rearrange("(p a) -> p a", p=FPC))
            nc.sync.dma_start(out=bv_sb[:], in_=bv_ext.ap().rearrange("(a f) -> a f", a=1))
            nc.sync.dma_start(out=bp_sb[:], in_=bp_ext.ap().rearrange("(a f) -> a f", a=1))

            wq_sb = wpool.tile([128, NEC, FPC], F32R)
            wk_sb = wpool.tile([128, NEC, FPC], F32R)
            wv_sb = wpool.tile([128, NEC, FPC], F32R)
            nc.sync.dma_start(out=wq_sb[:], in_=wq_ext.ap().rearrange("(j p) f -> p j f", p=128))
            nc.sync.dma_start(out=wk_sb[:], in_=wk_ext.ap().rearrange("(j p) f -> p j f", p=128))
            nc.sync.dma_start(out=wv_sb[:], in_=wv_ext.ap().rearrange("(j p) f -> p j f", p=128))

            qT = apool.tile([128, T], F32R)   # q features x all tokens
            kT = apool.tile([128, T], F32R)
            v_all = apool.tile([128, NTT, HPC, D + 1], F32R)  # [tok128, ktile, head, V|1]
            oT = apool.tile([128, T], F32R)   # attention out channels x tokens

            # ones column of v_all (softmax row-sum trick)
            nc.vector.tensor_copy(
                v_all[:, :, :, D : D + 1],
                ones_f32[:, 0 : NTT * HPC].rearrange("p (a b c) -> p a b c", a=NTT, b=HPC),
            )

            # ---------------- phase A+B: transpose + qkv projection ----------
            with (
                tc.tile_pool(name="xst", bufs=2) as xpool,
                tc.tile_pool(name="hT", bufs=2) as hpool,
                tc.tile_pool(name="ps_t", bufs=2, space="PSUM") as ps_t_pool,
                tc.tile_pool(name="ps_qk", bufs=2, space="PSUM") as ps_qk_pool,
                tc.tile_pool(name="ps_v", bufs=2, space="PSUM") as ps_v_pool,
            ):
                for st in range(NST):
                    x_t = xpool.tile([128, 4, E], F32, tag="x")
                    nc.sync.dma_start(
                        out=x_t[:],
                        in_=x_ext[st * 512 : (st + 1) * 512, :].rearrange(
                            "(i p) e -> p i e", p=128
                        ),
                    )
                    hT_st = hpool.tile([128, NEC, 512], F32R, tag="h")
                    for j in range(NEC):
                        ps_t = ps_t_pool.tile([128, 512], F32, tag="t")
                        for i in range(4):
                            nc.tensor.transpose(
                                ps_t[:, 128 * i : 128 * (i + 1)],
                                x_t[:, i, 128 * j : 128 * (j + 1)],
                                ident[:],
                            )
                        if j % 2 == 0:
                            nc.vector.tensor_copy(hT_st[:, j, :], ps_t[:])
                        else:
                            nc.scalar.activation(hT_st[:, j, :], ps_t[:], AF.Identity)
                    # qT / kT for this supertile
                    for w_sb, b_sb, dst in ((wq_sb, bq_sb, qT), (wk_sb, bk_sb, kT)):
                        ps = ps_qk_pool.tile([128, 512], F32, tag="qk")
                        for j in range(NEC):
                            nc.tensor.matmul(
                                ps[:],
                                w_sb[:, j, :],
                                hT_st[:, j, :],
                                start=(j == 0),
                                stop=(j == NEC - 1),
                            )
                        nc.scalar.activation(
                            dst[:, st * 512 : (st + 1) * 512],
                            ps[:],
                            AF.Identity,
                            bias=b_sb[:],
                        )
                    # V for the 4 token tiles of this supertile
                    for i in range(4):
                        tt = st * 4 + i
                        ps = ps_v_pool.tile([128, FPC], F32, tag="v")
                        for j in range(NEC):
                            nc.tensor.matmul(
                                ps[:],
                                hT_st[:, j, 128 * i : 128 * (i + 1)],
                                wv_sb[:, j, :],
                                start=(j == 0),
                                stop=False,
                            )
                        nc.tensor.matmul(
                            ps[:], ones_r[:, 0:128], bv_sb[:], start=False, stop=True
                        )
                        nc.vector.tensor_copy(
                            v_all[:, tt, :, 0:D],
                            ps[:].rearrange("p (h d) -> p h d", h=HPC),
                        )

            # ---------------- phase C: attention ----------------------------
            with (
                tc.tile_pool(name="pT", bufs=10) as ppool,
                tc.tile_pool(name="norm", bufs=3) as npool,
                tc.tile_pool(name="ps_s", bufs=3, space="PSUM") as ps_s_pool,
                tc.tile_pool(name="ps_o", bufs=2, space="PSUM") as ps_o_pool,
            ):
                for b in range(B):
                    for qt in range(QT_PER_B):
                        q0 = b * S + qt * 512
                        for h in range(HPC):
                            hp = 64 * h
                            pts = []
                            for ktp in range(KT_PER_B // 2):
                                ps_s = ps_s_pool.tile([128, 1024], F32, tag="s")
                                for i in range(2):
                                    kti = b * KT_PER_B + ktp * 2 + i
                                    nc.tensor.matmul(
                                        ps_s[:, 512 * i : 512 * (i + 1)],
                                        kT[hp : hp + 64, 128 * kti : 128 * (kti + 1)],
                                        qT[hp : hp + 64, q0 : q0 + 512],
                                        start=True,
                                        stop=True,
                                    )
                                pt = ppool.tile([128, 1024], F32R, tag="p")
                                nc.scalar.activation(pt[:], ps_s[:], AF.Exp, scale=0.125)
                                pts.append(pt)
                            ps_o = ps_o_pool.tile([128, 512], F32, tag="o")
                            for kt in range(KT_PER_B):
                                kti = b * KT_PER_B + kt
                                nc.tensor.matmul(
                                    ps_o[0 : D + 1, :],
                                    v_all[:, kti, h, :],
                                    pts[kt // 2][:, 512 * (kt % 2) : 512 * (kt % 2 + 1)],
                                    start=(kt == 0),
                                    stop=(kt == KT_PER_B - 1),
                                )
                            rec = npool.tile([1, 512], F32, tag="rec")
                            nc.vector.reciprocal(rec[:], ps_o[D : D + 1, :])
                            bc = npool.tile([64, 512], F32, tag="bc")
                            nc.gpsimd.partition_broadcast(bc[:], rec[:])
                            nc.vector.tensor_mul(
                                oT[hp : hp + 64, q0 : q0 + 512], ps_o[0:D, :], bc[:]
                            )

            # ---------------- A2A reshard ------------------------------------
            for j in range(NC):
                nc.sync.dma_start(
                    out=o_loc[j], in_=oT[:, TC * j : TC * (j + 1)]
                )
            nc.gpsimd.collective_compute(
                "AllToAll",
                mybir.AluOpType.bypass,
                replica_groups=[list(range(NC))],
                ins=[o_loc.ap().opt()],
                outs=[o_gat.ap().opt()],
            )

            # ---------------- phase D: output projection ---------------------
            with (
                tc.tile_pool(name="proj", bufs=1) as projpool,
                tc.tile_pool(name="ysb", bufs=3) as ypool,
                tc.tile_pool(name="ps_y", bufs=2, space="PSUM") as ps_y_pool,
            ):
                wp_sb = projpool.tile([128, NEC, E], F32R)
                nc.sync.dma_start(
                    out=wp_sb[:], in_=wp_ext.ap().rearrange("(j p) f -> p j f", p=128)
                )
                og = projpool.tile([128, NC, TC], F32R)
                nc.sync.dma_start(out=og[:], in_=o_gat.ap().rearrange("j p t -> p j t"))
                for ti in range(TC // 128):
                    for cb in range(E // 512):
                        ps_y = ps_y_pool.tile([128, 512], F32, tag="y")
                        for j in range(NEC):
                            nc.tensor.matmul(
                                ps_y[:],
                                og[:, j, 128 * ti : 128 * (ti + 1)],
                                wp_sb[:, j, 512 * cb : 512 * (cb + 1)],
                                start=(j == 0),
                                stop=False,
                            )
                        nc.tensor.matmul(
                            ps_y[:],
                            ones_r[:, 0:128],
                            bp_sb[:, 512 * cb : 512 * (cb + 1)],
                            start=False,
                            stop=True,
                        )
                        y_sb = ypool.tile([128, 512], F32, tag="ysb")
                        nc.vector.tensor_copy(y_sb[:], ps_y[:])
                        nc.sync.dma_start(
                            out=out_ext[
                                128 * ti : 128 * (ti + 1), 512 * cb : 512 * (cb + 1)
                            ],
                            in_=y_sb[:],
                        )

    nc.compile()
    return nc


_NC_CACHE = None


def _get_nc():
    global _NC_CACHE
    if _NC_CACHE is None:
        _NC_CACHE = build_nc()
    return _NC_CACHE


def kernel(
    hidden_states: np.ndarray,
    c_attn_w: np.ndarray,
    c_attn_b: np.ndarray,
    c_proj_w: np.ndarray,
    c_proj_b: np.ndarray,
    _want_results_obj: bool = False,
    **_unused,
) -> np.ndarray:
    x = np.ascontiguousarray(np.asarray(hidden_states, dtype=np.float32).reshape(T, E))
    w = np.asarray(c_attn_w, dtype=np.float32)
    battn = np.asarray(c_attn_b, dtype=np.float32)
    wp = np.ascontiguousarray(np.asarray(c_proj_w, dtype=np.float32))
    bp = np.asarray(c_proj_b, dtype=np.float32)

    in_maps = []
    for c in range(NC):
        f0 = FPC * c
        in_maps.append(
            {
                "x": x,
                "wq": np.ascontiguousarray(w[:, f0 : f0 + FPC]),
                "wk": np.ascontiguousarray(w[:, E + f0 : E + f0 + FPC]),
                "wv": np.ascontiguousarray(w[:, 2 * E + f0 : 2 * E + f0 + FPC]),
                "wp": wp,
                "bq": np.ascontiguousarray(battn[f0 : f0 + FPC]),
                "bk": np.ascontiguousarray(battn[E + f0 : E + f0 + FPC]),
                "bv": np.ascontiguousarray(battn[2 * E + f0 : 2 * E + f0 + FPC]),
                "bp": bp,
            }
        )

    nc = _get_nc()
    res = run_bass_kernel_spmd(nc, in_maps, core_ids=list(range(NC)))
    y = np.empty((T, E), dtype=np.float32)
    for c in range(NC):
        y[TC * c : TC * (c + 1)] = res.results[c]["out"]
    out = y.reshape(B, S, E)
    if _want_results_obj:
        return out, res
    return out
